# revision 22
# baseline (speedup 1.0000x reference)
"""Multi-head attention forward (B=4, L=2048, E=1024, H=16) on 8 NeuronCores.

Sharding: core c handles batch b = c // 2 and head-group g = c % 2 (8 heads,
512 embed dims). Each core computes its QKV projections, attention, and a
partial out-projection over its 512 contraction dims; the host sums the two
partials per batch and adds the bias.

Self-contained: only needs numpy + the concourse stack at /opt/trn_rl_repo.
"""

import os
import sys

import numpy as np

sys.path.insert(0, "/opt/trn_rl_repo")

import concourse.bass as bass  # noqa: E402
import concourse.tile as tile  # noqa: E402
from concourse import bacc, mybir  # noqa: E402
from concourse import bass_utils  # noqa: E402
from concourse.masks import make_identity  # noqa: E402

F32 = mybir.dt.float32
BF16 = mybir.dt.bfloat16
F32R = mybir.dt.float32r
EXP = mybir.ActivationFunctionType.Exp

P = 128          # partitions
L = 2048         # sequence length
E = 1024         # embed dim
FG = 512         # per-core feature slice (8 heads x 64)
D = 64           # head dim
LT = L // P      # 16 l-tiles
LG = L // 512    # 4 l-groups of 512
EC = E // P      # 8 e-chunks (contraction tiles for projections)
FT = FG // P     # 4 f-tiles (head pairs)
SC = L // P      # 16 s-chunks
GRP = 3          # score psum banks per exp group


def _build():
    nc = bacc.Bacc("TRN2", target_bir_lowering=False, debug=False, num_devices=8)

    debug = bool(os.environ.get("MHA_DEBUG"))
    xq_d = nc.dram_tensor("xq", [L, E], F32, kind="ExternalInput")
    xk_d = nc.dram_tensor("xk", [L, E], F32, kind="ExternalInput")
    xv_d = nc.dram_tensor("xv", [L, E], F32, kind="ExternalInput")
    wqkv_d = nc.dram_tensor("wqkv", [3 * FG, E], F32, kind="ExternalInput")
    wout_d = nc.dram_tensor("wout", [E, FG], F32, kind="ExternalInput")
    out_d = nc.dram_tensor("out", [L, E], F32, kind="ExternalOutput")
    if debug:
        dbg_qT = nc.dram_tensor("dbg_qT", [P, L], F32, kind="ExternalOutput")
        dbg_kT = nc.dram_tensor("dbg_kT", [P, L], F32, kind="ExternalOutput")
        dbg_v = nc.dram_tensor("dbg_v", [P, 256], F32, kind="ExternalOutput")
        dbg_sum = nc.dram_tensor("dbg_sum", [2, 512], F32, kind="ExternalOutput")
        dbg_avN = nc.dram_tensor("dbg_avN", [P, 512], F32, kind="ExternalOutput")

    with tile.TileContext(nc) as tc:
        with (
            tc.tile_pool(name="const", bufs=1) as constp,
            tc.tile_pool(name="qkv", bufs=1) as qkvp,
        ):
            warm32 = constp.tile([P, 16], F32, tag="warm32", name="warm32")
            nc.vector.memset(warm32[:], 0.0)
            warm16 = constp.tile([P, 16], BF16, tag="warm16", name="warm16")
            nc.vector.tensor_copy(warm16[:], warm32[:])  # DVE cast warm-up
            warmA = constp.tile([P, 16], F32, tag="warmA", name="warmA")
            nc.scalar.copy(warmA[:], warm32[:])          # ACT warm-up
            ident = constp.tile([P, P], F32, tag="ident", name="ident")
            make_identity(nc, ident[:])
            ident16 = constp.tile([P, P], BF16, tag="ident16", name="ident16")
            nc.vector.tensor_copy(ident16[:], ident[:])

            # persistent activation tensors
            qT = [qkvp.tile([P, L], F32R, tag=f"qT{i}", name=f"qT{i}")
                  for i in range(FT)]
            kT = [qkvp.tile([P, L], F32R, tag=f"kT{i}", name=f"kT{i}")
                  for i in range(FT)]
            # AV stationary tiles: per s-chunk, 4 pairs x 256 cols:
            #   [v_h0(64) | ones(1) | 0(63)]  -> av rows 0:64, sums row 64
            #   [0(32) | ones(1) | 0(31) | v_h1(64)] -> av rows 64:128, sums row 32
            vst = [qkvp.tile([P, 1024], BF16, tag=f"vst{i}", name=f"vst{i}")
                   for i in range(SC)]
            woutT = [qkvp.tile([P, E], F32R, tag=f"wo{ec}", name=f"wo{ec}")
                     for ec in range(4)]
            for scc in range(SC):
                nc.gpsimd.memset(vst[scc][:], 0.0)
                for pp in range(FT):
                    nc.gpsimd.memset(vst[scc][:, pp * 256 + 64: pp * 256 + 65], 1.0)
                    nc.gpsimd.memset(vst[scc][:, pp * 256 + 160: pp * 256 + 161], 1.0)

            # ---------------- phase 1: transposes + QKV projections -------
            with (
                tc.tile_pool(name="xT", bufs=1) as xTp,
                tc.tile_pool(name="wT", bufs=1) as wTp,
                tc.tile_pool(name="x32p", bufs=4) as x32p,
                tc.tile_pool(name="x16p", bufs=6) as x16p,
                tc.tile_pool(name="ps1", bufs=3, space="PSUM") as ps1,
            ):
                # weights: load natural [f, e] fp32, cast bf16, PE-transpose
                # into wT[ec] = [e-chunk 128, 1536] (cols: q 0:512, k, v)
                wT = [wTp.tile([P, 3 * FG], BF16, tag=f"wT{ec}", name=f"wT{ec}")
                      for ec in range(EC)]
                for i0 in (0, 4, 8):
                    w16s = []
                    for i in range(i0, i0 + 4):
                        w32 = x32p.tile([P, E], F32, tag="x32", name="w32")
                        nc.sync.dma_start(w32[:], wqkv_d.ap()[i * P:(i + 1) * P, :])
                        w16 = x16p.tile([P, E], BF16, tag="x16", name="w16")
                        nc.vector.tensor_copy(w16[:], w32[:])
                        w16s.append(w16)
                    for ec in range(EC):
                        tp = ps1.tile([P, 512], BF16, tag="tp", name="tp")
                        for j in range(4):
                            nc.tensor.transpose(
                                tp[:, j * P:(j + 1) * P],
                                w16s[j][:, ec * P:(ec + 1) * P],
                                ident16[:],
                            )
                        nc.scalar.copy(wT[ec][:, i0 * P:(i0 + 4) * P], tp[:])

                def load_and_project(xdram, kind, outT=None, wcol0=0):
                    # per 4-tile load group: transpose it, then immediately run
                    # the projection slice that depends only on this group
                    xT = [xTp.tile([P, L], BF16, tag=f"xT{ec}", name=f"xT{ec}")
                          for ec in range(EC)]
                    for lt0 in range(0, LT, 4):
                        x16s = []
                        for lt in range(lt0, lt0 + 4):
                            x32 = x32p.tile([P, E], F32, tag="x32", name="x32")
                            nc.sync.dma_start(
                                x32[:], xdram.ap()[lt * P:(lt + 1) * P, :])
                            x16 = x16p.tile([P, E], BF16, tag="x16", name="x16")
                            nc.vector.tensor_copy(x16[:], x32[:])
                            x16s.append(x16)
                        for ec in range(EC):
                            tp = ps1.tile([P, 512], BF16, tag="tp", name="tp")
                            for j in range(4):
                                nc.tensor.transpose(
                                    tp[:, j * P:(j + 1) * P],
                                    x16s[j][:, ec * P:(ec + 1) * P],
                                    ident16[:],
                                )
                            nc.scalar.copy(
                                xT[ec][:, lt0 * P:(lt0 + 4) * P], tp[:])
                        lg = lt0 // 4
                        if kind == "qk":
                            for ft in range(FT):
                                ps = ps1.tile([P, 512], F32, tag="mm1",
                                              name="mm1")
                                for ec in range(EC):
                                    nc.tensor.matmul(
                                        ps[:],
                                        wT[ec][:, wcol0 + ft * P:
                                               wcol0 + (ft + 1) * P],
                                        xT[ec][:, lg * 512:(lg + 1) * 512],
                                        start=(ec == 0),
                                        stop=(ec == EC - 1),
                                    )
                                    if ec == 0 and ft == 0:
                                        pass
                                nc.vector.tensor_copy(
                                    outT[ft][:, lg * 512:(lg + 1) * 512], ps[:])
                        else:  # v: natural layout into padded stationaries
                            for lt in range(lt0, lt0 + 4):
                                ps = ps1.tile([P, 512], F32, tag="mm1",
                                              name="mm1")
                                for ec in range(EC):
                                    nc.tensor.matmul(
                                        ps[:],
                                        xT[ec][:, lt * P:(lt + 1) * P],
                                        wT[ec][:, 2 * FG:3 * FG],
                                        start=(ec == 0),
                                        stop=(ec == EC - 1),
                                    )
                                for pp in range(FT):
                                    nc.vector.tensor_copy(
                                        vst[lt][:, pp * 256: pp * 256 + 64],
                                        ps[:, pp * P: pp * P + 64])
                                    nc.vector.tensor_copy(
                                        vst[lt][:, pp * 256 + 192:
                                                pp * 256 + 256],
                                        ps[:, pp * P + 64: pp * P + 128])

                # wout transposes (f32r, fp32 identity) while x loads stream
                for i0 in (0, 4):
                    w32s = []
                    for i in range(i0, i0 + 4):
                        w32 = x32p.tile([P, FG], F32, tag="w32o", name="w32o",
                                        bufs=5)
                        nc.sync.dma_start(
                            w32[:], wout_d.ap()[i * P:(i + 1) * P, :])
                        w32s.append(w32)
                    for ec in range(4):
                        tp3 = ps1.tile([P, 512], F32, tag="tp3", name="tp3",
                                       bufs=2)
                        for j in range(4):
                            nc.tensor.transpose(
                                tp3[:, j * P:(j + 1) * P],
                                w32s[j][:, ec * P:(ec + 1) * P],
                                ident[:],
                            )
                        nc.vector.tensor_copy(
                            woutT[ec][:, i0 * P:(i0 + 4) * P], tp3[:])

                load_and_project(xv_d, "v")
                load_and_project(xq_d, "qk", qT, 0)
                load_and_project(xk_d, "qk", kT, FG)

                if debug:
                    nc.sync.dma_start(dbg_qT.ap(), qT[0][:].bitcast(F32))
                    nc.sync.dma_start(dbg_kT.ap(), kT[0][:].bitcast(F32))
                    vstg = x32p.tile([P, 256], F32, tag="x32", name="vstg")
                    nc.vector.tensor_copy(vstg[:], vst[0][:, 0:256])
                    nc.sync.dma_start(dbg_v.ap(), vstg[:])

            # ---------------- phase 2: attention --------------------------
            with tc.tile_pool(name="avN", bufs=1) as avNp:
                avN = [avNp.tile([P, L], F32R, tag=f"avN{i}", name=f"avN{i}")
                       for i in range(FT)]
                with (
                    tc.tile_pool(name="attnT", bufs=3) as attp,
                    tc.tile_pool(name="srow", bufs=4) as srow,
                    tc.tile_pool(name="bc", bufs=2) as bcp,
                    tc.tile_pool(name="ps_sc", bufs=2, space="PSUM") as ps_sc,
                    tc.tile_pool(name="ps_av", bufs=1, space="PSUM") as ps_av,
                ):
                    for p in range(FT):
                        for lg in range(LG):
                            avA = ps_av.tile([P, 512], F32, tag="avA", name="avA")
                            avB = ps_av.tile([P, 512], F32, tag="avB", name="avB")
                            av_bank = (avA, avB)

                            def av_mms(t0, n, aT, p=p, av_bank=av_bank):
                                for j in range(n):
                                    sc, h = divmod(t0 + j, 2)
                                    nc.tensor.matmul(
                                        av_bank[h][:],
                                        vst[sc][:, p * 256 + 128 * h:
                                                p * 256 + 128 * h + 128],
                                        aT[:, j * 512:(j + 1) * 512],
                                        start=(sc == 0),
                                        stop=(sc == SC - 1),
                                    )

                            pending = None
                            for t0 in range(0, 2 * SC, GRP):
                                n = min(GRP, 2 * SC - t0)
                                sc_ps = ps_sc.tile([P, 512 * n], F32, tag="sc",
                                                   name="sc")
                                for j in range(n):
                                    sc, h = divmod(t0 + j, 2)
                                    nc.tensor.matmul(
                                        sc_ps[:, j * 512:(j + 1) * 512],
                                        kT[p][64 * h:64 * h + 64,
                                              sc * P:(sc + 1) * P],
                                        qT[p][64 * h:64 * h + 64,
                                              lg * 512:(lg + 1) * 512],
                                        start=True,
                                        stop=True,
                                    )
                                aT = attp.tile([P, 512 * n], BF16, tag="aT",
                                               name="aT")
                                nc.scalar.activation(aT[:], sc_ps[:], EXP,
                                                     scale=0.125)
                                if pending is not None:
                                    av_mms(*pending)
                                pending = (t0, n, aT)
                            av_mms(*pending)

                            # drain av banks to SBUF fast (frees PSUM),
                            # then normalize off the critical path
                            avS0 = bcp.tile([P, 512], F32, tag="avS0",
                                            name="avS0")
                            nc.vector.tensor_copy(avS0[:], avA[:])
                            avS1 = bcp.tile([P, 512], F32, tag="avS1",
                                            name="avS1")
                            nc.vector.tensor_copy(avS1[:], avB[:])
                            r0 = srow.tile([1, 512], F32, tag="r0", name="r0")
                            nc.vector.reciprocal(r0[:], avS0[64:65, :])
                            r1 = srow.tile([1, 512], F32, tag="r1", name="r1")
                            nc.vector.reciprocal(r1[:], avS1[32:33, :])
                            bc0 = bcp.tile([P, 512], F32, tag="bc0", name="bc0")
                            nc.gpsimd.partition_broadcast(bc0[:], r0[:])
                            bc1 = bcp.tile([P, 512], F32, tag="bc1", name="bc1")
                            nc.gpsimd.partition_broadcast(bc1[:], r1[:])
                            nc.vector.tensor_mul(
                                avN[p][0:64, lg * 512:(lg + 1) * 512],
                                avS0[0:64, :], bc0[0:64, :])
                            nc.vector.tensor_mul(
                                avN[p][64:128, lg * 512:(lg + 1) * 512],
                                avS1[64:128, :], bc1[64:128, :])
                            if debug and p == 0 and lg == 0:
                                nc.sync.dma_start(dbg_sum.ap()[0:1, :],
                                                  avS0[64:65, :])
                                nc.sync.dma_start(dbg_sum.ap()[1:2, :],
                                                  avS1[32:33, :])
                                nc.sync.dma_start(
                                    dbg_avN.ap(),
                                    avN[0][:, 0:512].bitcast(F32))

                # ------------ phase 3: output projection ------------------
                with (
                    tc.tile_pool(name="ost", bufs=3) as ost,
                    tc.tile_pool(name="ps3", bufs=4, space="PSUM") as ps3,
                ):
                    for lt in range(LT):
                        osb = ost.tile([P, E], F32, tag="osb", name="osb")
                        psA = ps3.tile([P, 512], F32, tag="mm3", name="psA")
                        psB = ps3.tile([P, 512], F32, tag="mm3", name="psB")
                        for ec in range(4):
                            # both fg halves back-to-back: stationary reused
                            nc.tensor.matmul(
                                psA[:], avN[ec][:, lt * P:(lt + 1) * P],
                                woutT[ec][:, 0:512],
                                start=(ec == 0), stop=(ec == 3))
                            nc.tensor.matmul(
                                psB[:], avN[ec][:, lt * P:(lt + 1) * P],
                                woutT[ec][:, 512:1024],
                                start=(ec == 0), stop=(ec == 3))
                        nc.scalar.copy(osb[:, 0:512], psA[:])
                        nc.vector.tensor_copy(osb[:, 512:1024], psB[:])
                        nc.sync.dma_start(
                            out_d.ap()[lt * P:(lt + 1) * P, :], osb[:])

    nc.compile()
    return nc


_NC = None


def _get_nc():
    global _NC
    if _NC is None:
        _NC = _build()
    return _NC


def _shard_inputs(query, key, value, in_proj_weight, out_proj_weight):
    in_maps = []
    for c in range(8):
        b, g = divmod(c, 2)
        sl = slice(FG * g, FG * g + FG)
        wq = in_proj_weight[0 * E:0 * E + E][sl]
        wk = in_proj_weight[1 * E:1 * E + E][sl]
        wv = in_proj_weight[2 * E:2 * E + E][sl]
        in_maps.append({
            "xq": np.ascontiguousarray(query[b], dtype=np.float32),
            "xk": np.ascontiguousarray(key[b], dtype=np.float32),
            "xv": np.ascontiguousarray(value[b], dtype=np.float32),
            "wqkv": np.ascontiguousarray(
                np.concatenate([wq, wk, wv], axis=0), dtype=np.float32),
            "wout": np.ascontiguousarray(out_proj_weight[:, sl], dtype=np.float32),
        })
    return in_maps


def run_sharded(in_maps, **kwargs):
    nc = _get_nc()
    return bass_utils.run_bass_kernel_spmd(
        nc, in_maps, core_ids=list(range(8)), **kwargs)


def kernel(query, key, value, in_proj_weight, out_proj_weight, out_proj_bias):
    query = np.asarray(query, dtype=np.float32)
    key = np.asarray(key, dtype=np.float32)
    value = np.asarray(value, dtype=np.float32)
    in_proj_weight = np.asarray(in_proj_weight, dtype=np.float32)
    out_proj_weight = np.asarray(out_proj_weight, dtype=np.float32)
    out_proj_bias = np.asarray(out_proj_bias, dtype=np.float32)

    in_maps = _shard_inputs(query, key, value, in_proj_weight, out_proj_weight)
    res = run_sharded(in_maps)
    out = np.empty((4, L, E), dtype=np.float32)
    for b in range(4):
        out[b] = res.results[2 * b]["out"] + res.results[2 * b + 1]["out"]
    out += out_proj_bias
    return out
